# revision 6
# baseline (speedup 1.0000x reference)
"""Trainium2 Bass kernel for quantized BERT self-attention (ternary QKV).

Math notes (exact, input-independent):
  - sym_quantize(x, 2.5, 2) == 2.5 * t(x) with t(x) = ternary {-1,0,+1},
    thresholds at +-1.25.
  - softmax probs lie in (0, 1], and round(p/2.5) == 0 for all p <= 1.25,
    so probs_q == 0 identically => context_layer and context_layer_ are
    exactly zero. They are produced host-side; the device computes scores,
    attention probs and v_q.

Sharding: data-parallel over batch, 2 batches per core on 8 cores.
"""

import sys
from contextlib import ExitStack

for _p in ("/opt/trn_rl_repo", "/root/.axon_site/_ro/trn_rl_repo"):
    if _p not in sys.path:
        sys.path.insert(0, _p)

import numpy as np
import ml_dtypes

import concourse.bass as bass
import concourse.tile as tile
from concourse import bacc, mybir
from concourse import bass_utils

BF16 = ml_dtypes.bfloat16
F32 = mybir.dt.float32
BF = mybir.dt.bfloat16
AF = mybir.ActivationFunctionType

N_CORES = 8
B, S, H = 16, 512, 1024
NH, D = 16, 64
BPC = B // N_CORES          # batches per core (2)
NSTRIP = H // 128           # 8 strips of 128 output features
NKT = H // 128              # 8 contraction tiles
NQT = S // 128              # 4 query-row tiles per head
SCALE = 2.5 * 2.5 / (4.0 * np.sqrt(D))   # 0.1953125: psum holds 4*unit-dot

_CACHE = {}


def _build(bias_nz: bool, mask_nz: bool):
    nc = bacc.Bacc("TRN2", target_bir_lowering=False, debug=False,
                   enable_asserts=False, num_devices=N_CORES)

    # ---- DRAM I/O (per core) ----
    hiT_d = nc.dram_tensor("hiT", [BPC, H, S], BF, kind="ExternalInput").ap()
    loT_d = nc.dram_tensor("loT", [BPC, H, S], BF, kind="ExternalInput").ap()
    wqh_d = nc.dram_tensor("wqh", [H, H], BF, kind="ExternalInput").ap()
    wql_d = nc.dram_tensor("wql", [H, H], BF, kind="ExternalInput").ap()
    wkh_d = nc.dram_tensor("wkh", [H, H], BF, kind="ExternalInput").ap()
    wkl_d = nc.dram_tensor("wkl", [H, H], BF, kind="ExternalInput").ap()
    wvh_d = nc.dram_tensor("wvh", [H, H], BF, kind="ExternalInput").ap()
    wvl_d = nc.dram_tensor("wvl", [H, H], BF, kind="ExternalInput").ap()
    # per-out-feature sign thresholds: (bq-1.25, bq+1.25, bk-1.25, bk+1.25)
    thr_d = nc.dram_tensor("thr", [H, 4], F32, kind="ExternalInput").ap()
    if bias_nz:
        bvh_d = nc.dram_tensor("bvh", [1, H], BF, kind="ExternalInput").ap()
        bvl_d = nc.dram_tensor("bvl", [1, H], BF, kind="ExternalInput").ap()
    if mask_nz:
        # mask replicated across partitions host-side: [BPC, 128, S]
        msk_d = nc.dram_tensor("msk", [BPC, 128, S], F32, kind="ExternalInput").ap()

    sc_d = nc.dram_tensor("scores", [BPC, NH, S, S], F32, kind="ExternalOutput").ap()
    pr_d = nc.dram_tensor("probs", [BPC, NH, S, S], F32, kind="ExternalOutput").ap()
    vq_d = nc.dram_tensor("vq", [BPC, NH, S, D], F32, kind="ExternalOutput").ap()

    with tile.TileContext(nc) as tc, ExitStack() as ctx:
        hid_p = ctx.enter_context(tc.tile_pool(name="hid", bufs=1))
        tern_p = ctx.enter_context(tc.tile_pool(name="tern", bufs=1))
        w_p = ctx.enter_context(tc.tile_pool(name="w", bufs=2))
        wv_p = ctx.enter_context(tc.tile_pool(name="wv", bufs=1))
        thr_p = ctx.enter_context(tc.tile_pool(name="thr", bufs=3))
        ab_p = ctx.enter_context(tc.tile_pool(name="ab", bufs=3))
        sc_p = ctx.enter_context(tc.tile_pool(name="sc", bufs=2))
        p_p = ctx.enter_context(tc.tile_pool(name="p", bufs=2))
        vq_p = ctx.enter_context(tc.tile_pool(name="vqs", bufs=2))
        tv_p = ctx.enter_context(tc.tile_pool(name="tv", bufs=4))
        sum_p = ctx.enter_context(tc.tile_pool(name="sum", bufs=4))
        const_p = ctx.enter_context(tc.tile_pool(name="const", bufs=1))
        msk_p = ctx.enter_context(tc.tile_pool(name="msk", bufs=1))

        ps_proj = ctx.enter_context(tc.tile_pool(name="ps_proj", bufs=2, space="PSUM"))
        ps_sc = ctx.enter_context(tc.tile_pool(name="ps_sc", bufs=2, space="PSUM"))
        ps_v = ctx.enter_context(tc.tile_pool(name="ps_v", bufs=2, space="PSUM"))

        # constants
        c_m = const_p.tile([128, 1], F32, tag="c_m")
        c_p = const_p.tile([128, 1], F32, tag="c_p")
        nc.vector.memset(c_m[:], -1.25)
        nc.vector.memset(c_p[:], 1.25)
        if bias_nz:
            ones_t = const_p.tile([1, 128], BF, tag="ones")
            nc.vector.memset(ones_t[:], 1.0)
            bvh_t = const_p.tile([1, H], BF, tag="bvh")
            bvl_t = const_p.tile([1, H], BF, tag="bvl")
            nc.sync.dma_start(bvh_t[:], bvh_d[:])
            nc.sync.dma_start(bvl_t[:], bvl_d[:])

        # resident transposed hidden halves: [128, NKT, S] per (b, hi/lo)
        hiT = []
        loT = []
        for b in range(BPC):
            th = hid_p.tile([128, NKT, S], BF, tag=f"hiT{b}")
            tl = hid_p.tile([128, NKT, S], BF, tag=f"loT{b}")
            nc.sync.dma_start(th[:], hiT_d[b].rearrange("(t p) n -> p t n", p=128))
            nc.sync.dma_start(tl[:], loT_d[b].rearrange("(t p) n -> p t n", p=128))
            hiT.append(th)
            loT.append(tl)

        if mask_nz:
            msk = []
            for b in range(BPC):
                mt = msk_p.tile([128, S], F32, tag=f"msk{b}")
                nc.sync.dma_start(mt[:], msk_d[b])
                msk.append(mt)

        tq = [[None] * NSTRIP for _ in range(BPC)]
        tk = [[None] * NSTRIP for _ in range(BPC)]

        def proj_psum(ps, wh, wl, b):
            """accumulate x @ W for strip: 3-term bf16 split, W stationary."""
            for kt in range(NKT):
                st = kt == 0
                nc.tensor.matmul(ps[:], wh[:, kt, :], hiT[b][:, kt, :],
                                 start=st, stop=False)
                nc.tensor.matmul(ps[:], wh[:, kt, :], loT[b][:, kt, :],
                                 start=False, stop=False)
                nc.tensor.matmul(ps[:], wl[:, kt, :], hiT[b][:, kt, :],
                                 start=False, stop=(kt == NKT - 1))

        def ternarize(ps, lo_thr, hi_thr, out_bf):
            """out = Sign(ps + lo_thr) + Sign(ps + hi_thr)  in {-2,0,2} bf16."""
            a = ab_p.tile([128, S], BF, tag="a")
            bb = ab_p.tile([128, S], BF, tag="b")
            nc.scalar.activation(a[:], ps[:], AF.Sign, bias=lo_thr)
            nc.scalar.activation(bb[:], ps[:], AF.Sign, bias=hi_thr)
            nc.vector.tensor_add(out_bf[:], a[:], bb[:])

        # ---------- Phase A: q/k projections + quantize, per strip ----------
        for s in range(NSTRIP):
            wqh = w_p.tile([128, NKT, 128], BF, tag="wqh")
            wql = w_p.tile([128, NKT, 128], BF, tag="wql")
            wkh = w_p.tile([128, NKT, 128], BF, tag="wkh")
            wkl = w_p.tile([128, NKT, 128], BF, tag="wkl")
            for t_, d_ in ((wqh, wqh_d), (wql, wql_d), (wkh, wkh_d), (wkl, wkl_d)):
                nc.sync.dma_start(
                    t_[:], d_[:, s * 128:(s + 1) * 128].rearrange("(t p) m -> p t m", p=128))
            thr_t = thr_p.tile([128, 4], F32, tag="thr")
            nc.sync.dma_start(thr_t[:], thr_d[s * 128:(s + 1) * 128, :])

            for b in range(BPC):
                psq = ps_proj.tile([128, S], F32, tag="psq")
                proj_psum(psq, wqh, wql, b)
                tqt = tern_p.tile([128, S], BF, tag=f"tq{b}_{s}")
                ternarize(psq, thr_t[:, 0:1], thr_t[:, 1:2], tqt)
                tq[b][s] = tqt

                psk = ps_proj.tile([128, S], F32, tag="psk")
                proj_psum(psk, wkh, wkl, b)
                tkt = tern_p.tile([128, S], BF, tag=f"tk{b}_{s}")
                ternarize(psk, thr_t[:, 2:3], thr_t[:, 3:4], tkt)
                tk[b][s] = tkt

            # ------- scores + softmax for the 2 heads of this strip -------
            for b in range(BPC):
                for hh in range(2):
                    h = 2 * s + hh
                    off = hh * D
                    sc_t = sc_p.tile([128, NQT, S], F32, tag="sc")
                    p_t = p_p.tile([128, NQT, S], F32, tag="p")
                    ssum = sum_p.tile([128, NQT], F32, tag="ssum")
                    rinv = sum_p.tile([128, NQT], F32, tag="rinv")
                    for m in range(NQT):
                        pss = ps_sc.tile([128, S], F32, tag="pss")
                        nc.tensor.matmul(
                            pss[:],
                            tq[b][s][off:off + D, m * 128:(m + 1) * 128],
                            tk[b][s][off:off + D, :],
                            start=True, stop=True)
                        if mask_nz:
                            # scores = SCALE*psum + mask ; exp from SBUF scores
                            nc.vector.tensor_scalar(
                                sc_t[:, m, :], pss[:], SCALE, None,
                                mybir.AluOpType.mult)
                            nc.vector.tensor_add(sc_t[:, m, :], sc_t[:, m, :],
                                                 msk[b][:])
                            nc.scalar.activation(
                                p_t[:, m, :], sc_t[:, m, :], AF.Exp,
                                accum_out=ssum[:, m:m + 1])
                        else:
                            nc.vector.tensor_scalar(
                                sc_t[:, m, :], pss[:], SCALE, None,
                                mybir.AluOpType.mult)
                            nc.scalar.activation(
                                p_t[:, m, :], pss[:], AF.Exp, scale=SCALE,
                                accum_out=ssum[:, m:m + 1])
                    nc.vector.reciprocal(rinv[:], ssum[:])
                    for m in range(NQT):
                        nc.vector.tensor_scalar(
                            p_t[:, m, :], p_t[:, m, :], rinv[:, m:m + 1], None,
                            mybir.AluOpType.mult)
                    dview = sc_d[b, h].rearrange("(m p) k -> p m k", p=128)
                    nc.sync.dma_start(dview, sc_t[:])
                    nc.sync.dma_start(
                        pr_d[b, h].rearrange("(m p) k -> p m k", p=128), p_t[:])

        # ---------- Phase V: v token-major + quantize ----------
        for nh in range(2):
            wvh = wv_p.tile([128, NKT, 512], BF, tag="wvh")
            wvl = wv_p.tile([128, NKT, 512], BF, tag="wvl")
            nc.sync.dma_start(
                wvh[:], wvh_d[:, nh * 512:(nh + 1) * 512].rearrange(
                    "(t p) m -> p t m", p=128))
            nc.sync.dma_start(
                wvl[:], wvl_d[:, nh * 512:(nh + 1) * 512].rearrange(
                    "(t p) m -> p t m", p=128))
            for b in range(BPC):
                vq_t = vq_p.tile([128, NQT, 512], F32, tag="vq")
                for mt in range(NQT):
                    psv = ps_v.tile([128, 512], F32, tag="psv")
                    sl = slice(mt * 128, (mt + 1) * 128)
                    for kt in range(NKT):
                        nc.tensor.matmul(psv[:], hiT[b][:, kt, sl], wvh[:, kt, :],
                                         start=(kt == 0), stop=False)
                        nc.tensor.matmul(psv[:], hiT[b][:, kt, sl], wvl[:, kt, :],
                                         start=False, stop=False)
                        last = (kt == NKT - 1) and not bias_nz
                        nc.tensor.matmul(psv[:], loT[b][:, kt, sl], wvh[:, kt, :],
                                         start=False, stop=last)
                    if bias_nz:
                        nc.tensor.matmul(psv[:], ones_t[:],
                                         bvh_t[:, nh * 512:(nh + 1) * 512],
                                         start=False, stop=False)
                        nc.tensor.matmul(psv[:], ones_t[:],
                                         bvl_t[:, nh * 512:(nh + 1) * 512],
                                         start=False, stop=True)
                    tv = tv_p.tile([128, 512], BF, tag="tv")
                    ternarize(psv, c_m[:], c_p[:], tv)
                    nc.vector.tensor_scalar(vq_t[:, mt, :], tv[:], 1.25, None,
                                            mybir.AluOpType.mult)
                # [128, mt, (h d)] -> dram [b, h(8), s(m,p), d], one DMA per mt
                for mt in range(NQT):
                    dview = vq_d[b, nh * 8:(nh + 1) * 8,
                                 mt * 128:(mt + 1) * 128, :].rearrange(
                                     "h p d -> p h d")
                    nc.sync.dma_start(
                        dview, vq_t[:, mt, :].rearrange("p (h d) -> p h d", h=8))

    nc.compile()
    return nc


def _get_nc(bias_nz: bool, mask_nz: bool):
    key = (bias_nz, mask_nz)
    if key not in _CACHE:
        _CACHE[key] = _build(bias_nz, mask_nz)
    return _CACHE[key]


def _split_hi_lo(x32):
    hi = x32.astype(BF16)
    lo = (x32 - hi.astype(np.float32)).astype(BF16)
    return hi, lo


def _prep(hidden_states, attention_mask, Wq, bq, Wk, bk, Wv, bv):
    hs = np.asarray(hidden_states, np.float32)
    msk = np.asarray(attention_mask, np.float32)
    Wq = np.asarray(Wq, np.float32)
    Wk = np.asarray(Wk, np.float32)
    Wv = np.asarray(Wv, np.float32)
    bq = np.asarray(bq, np.float32)
    bk = np.asarray(bk, np.float32)
    bv = np.asarray(bv, np.float32)

    bias_nz = bool(np.any(bq) or np.any(bk) or np.any(bv))
    mask_nz = bool(np.any(msk))

    # transposed hidden, hi/lo split: [B, H, S]
    hsT = np.ascontiguousarray(hs.transpose(0, 2, 1))
    hiT, loT = _split_hi_lo(hsT)
    wqh, wql = _split_hi_lo(Wq)
    wkh, wkl = _split_hi_lo(Wk)
    wvh, wvl = _split_hi_lo(Wv)
    thr = np.stack([bq - 1.25, bq + 1.25, bk - 1.25, bk + 1.25],
                   axis=1).astype(np.float32)

    shared = {
        "wqh": np.ascontiguousarray(wqh), "wql": np.ascontiguousarray(wql),
        "wkh": np.ascontiguousarray(wkh), "wkl": np.ascontiguousarray(wkl),
        "wvh": np.ascontiguousarray(wvh), "wvl": np.ascontiguousarray(wvl),
        "thr": thr,
    }
    if bias_nz:
        bvh, bvl = _split_hi_lo(bv)
        shared["bvh"] = np.ascontiguousarray(bvh[None, :])
        shared["bvl"] = np.ascontiguousarray(bvl[None, :])

    in_maps = []
    for c in range(N_CORES):
        bsl = slice(c * BPC, (c + 1) * BPC)
        m = dict(shared)
        m["hiT"] = np.ascontiguousarray(hiT[bsl])
        m["loT"] = np.ascontiguousarray(loT[bsl])
        if mask_nz:
            mm = msk[bsl, 0, 0, :]                      # [BPC, S]
            m["msk"] = np.ascontiguousarray(
                np.broadcast_to(mm[:, None, :], (BPC, 128, S)).astype(np.float32))
        in_maps.append(m)
    return in_maps, bias_nz, mask_nz


def _run(hidden_states, attention_mask, Wq, bq, Wk, bk, Wv, bv, trace=False):
    in_maps, bias_nz, mask_nz = _prep(hidden_states, attention_mask,
                                      Wq, bq, Wk, bk, Wv, bv)
    nc = _get_nc(bias_nz, mask_nz)
    res = bass_utils.run_bass_kernel_spmd(
        nc, in_maps, core_ids=list(range(N_CORES)), trace=trace)

    scores = np.concatenate([res.results[c]["scores"] for c in range(N_CORES)], 0)
    probs = np.concatenate([res.results[c]["probs"] for c in range(N_CORES)], 0)
    vq = np.concatenate([res.results[c]["vq"] for c in range(N_CORES)], 0)

    context_layer = np.zeros((B, S, H), np.float32)
    context_layer_ = np.zeros((B, NH, S, D), np.float32)
    out = (context_layer, scores, probs, context_layer_, vq)
    return (out, res) if trace else out


def kernel(hidden_states, attention_mask, Wq, bq, Wk, bk, Wv, bv):
    return _run(hidden_states, attention_mask, Wq, bq, Wk, bk, Wv, bv)


# revision 10
# speedup vs baseline: 1.1356x; 1.1356x over previous
"""Trainium2 Bass kernel for quantized BERT self-attention (ternary QKV).

Math notes (exact, input-independent):
  - sym_quantize(x, 2.5, 2) == 2.5 * t(x) with t(x) = ternary {-1,0,+1},
    thresholds at +-1.25.
  - softmax probs lie in (0, 1], and round(p/2.5) == 0 for all p <= 1.25,
    so probs_q == 0 identically => context_layer and context_layer_ are
    exactly zero. They are produced host-side; the device computes scores,
    attention probs and v_q.

Sharding: data-parallel over batch, 2 batches per core on 8 cores.
"""

import sys
from contextlib import ExitStack

for _p in ("/opt/trn_rl_repo", "/root/.axon_site/_ro/trn_rl_repo"):
    if _p not in sys.path:
        sys.path.insert(0, _p)

import numpy as np
import ml_dtypes

import concourse.bass as bass
import concourse.tile as tile
from concourse import bacc, mybir
from concourse import bass_utils

BF16 = ml_dtypes.bfloat16
F32 = mybir.dt.float32
BF = mybir.dt.bfloat16
AF = mybir.ActivationFunctionType

N_CORES = 8
B, S, H = 16, 512, 1024
NH, D = 16, 64
BPC = B // N_CORES          # batches per core (2)
NSTRIP = H // 128           # 8 strips of 128 output features
NKT = H // 128              # 8 contraction tiles
NQT = S // 128              # 4 query-row tiles per head
SCALE = 2.5 * 2.5 / (4.0 * np.sqrt(D))   # 0.1953125: psum holds 4*unit-dot

_CACHE = {}


def _build(bias_nz: bool, mask_nz: bool):
    nc = bacc.Bacc("TRN2", target_bir_lowering=False, debug=False,
                   enable_asserts=False, num_devices=N_CORES)

    # ---- DRAM I/O (per core) ----
    hiT_d = nc.dram_tensor("hiT", [BPC, H, S], BF, kind="ExternalInput").ap()
    loT_d = nc.dram_tensor("loT", [BPC, H, S], BF, kind="ExternalInput").ap()
    wqh_d = nc.dram_tensor("wqh", [H, H], BF, kind="ExternalInput").ap()
    wql_d = nc.dram_tensor("wql", [H, H], BF, kind="ExternalInput").ap()
    wkh_d = nc.dram_tensor("wkh", [H, H], BF, kind="ExternalInput").ap()
    wkl_d = nc.dram_tensor("wkl", [H, H], BF, kind="ExternalInput").ap()
    wvh_d = nc.dram_tensor("wvh", [H, H], BF, kind="ExternalInput").ap()
    wvl_d = nc.dram_tensor("wvl", [H, H], BF, kind="ExternalInput").ap()
    # per-out-feature sign thresholds: (bq-1.25, bq+1.25, bk-1.25, bk+1.25)
    thr_d = nc.dram_tensor("thr", [H, 4], F32, kind="ExternalInput").ap()
    if bias_nz:
        bvh_d = nc.dram_tensor("bvh", [1, H], BF, kind="ExternalInput").ap()
        bvl_d = nc.dram_tensor("bvl", [1, H], BF, kind="ExternalInput").ap()
    if mask_nz:
        # mask replicated across partitions host-side: [BPC, 128, S]
        msk_d = nc.dram_tensor("msk", [BPC, 128, S], F32, kind="ExternalInput").ap()

    sc_d = nc.dram_tensor("scores", [BPC, NH, S, S], F32, kind="ExternalOutput").ap()
    pr_d = nc.dram_tensor("probs", [BPC, NH, S, S], F32, kind="ExternalOutput").ap()
    vq_d = nc.dram_tensor("vq", [BPC, NH, S, D], F32, kind="ExternalOutput").ap()

    with tile.TileContext(nc) as tc, ExitStack() as ctx:
        hid_p = ctx.enter_context(tc.tile_pool(name="hid", bufs=1))
        tern_p = ctx.enter_context(tc.tile_pool(name="tern", bufs=1))
        w_p = ctx.enter_context(tc.tile_pool(name="w", bufs=2))
        wv_p = ctx.enter_context(tc.tile_pool(name="wv", bufs=2))
        thr_p = ctx.enter_context(tc.tile_pool(name="thr", bufs=3))
        ab_p = ctx.enter_context(tc.tile_pool(name="ab", bufs=3))
        sc_p = ctx.enter_context(tc.tile_pool(name="sc", bufs=2))
        p_p = ctx.enter_context(tc.tile_pool(name="p", bufs=2))
        vq_p = ctx.enter_context(tc.tile_pool(name="vqs", bufs=4))
        tv_p = ctx.enter_context(tc.tile_pool(name="tv", bufs=4))
        sum_p = ctx.enter_context(tc.tile_pool(name="sum", bufs=4))
        const_p = ctx.enter_context(tc.tile_pool(name="const", bufs=1))
        msk_p = ctx.enter_context(tc.tile_pool(name="msk", bufs=1))

        ps_proj = ctx.enter_context(tc.tile_pool(name="ps_proj", bufs=3, space="PSUM"))
        ps_sc = ctx.enter_context(tc.tile_pool(name="ps_sc", bufs=3, space="PSUM"))
        ps_v = ctx.enter_context(tc.tile_pool(name="ps_v", bufs=2, space="PSUM"))

        # constants
        c_m = const_p.tile([128, 1], F32, tag="c_m")
        c_p = const_p.tile([128, 1], F32, tag="c_p")
        nc.vector.memset(c_m[:], -1.25)
        nc.vector.memset(c_p[:], 1.25)
        if bias_nz:
            ones_t = const_p.tile([1, 128], BF, tag="ones")
            nc.vector.memset(ones_t[:], 1.0)
            bvh_t = const_p.tile([1, H], BF, tag="bvh")
            bvl_t = const_p.tile([1, H], BF, tag="bvl")
            nc.sync.dma_start(bvh_t[:], bvh_d[:])
            nc.sync.dma_start(bvl_t[:], bvl_d[:])

        def load_w_strip(s):
            wqh = w_p.tile([128, NKT, 128], BF, tag="wqh")
            wql = w_p.tile([128, NKT, 128], BF, tag="wql")
            wkh = w_p.tile([128, NKT, 128], BF, tag="wkh")
            wkl = w_p.tile([128, NKT, 128], BF, tag="wkl")
            for t_, d_ in ((wqh, wqh_d), (wql, wql_d), (wkh, wkh_d), (wkl, wkl_d)):
                nc.sync.dma_start(
                    t_[:], d_[:, s * 128:(s + 1) * 128].rearrange(
                        "(t p) m -> p t m", p=128))
            thr_t = thr_p.tile([128, 4], F32, tag="thr")
            nc.sync.dma_start(thr_t[:], thr_d[s * 128:(s + 1) * 128, :])
            return wqh, wql, wkh, wkl, thr_t

        def load_wv(nh):
            wvh = wv_p.tile([128, NKT, 512], BF, tag="wvh")
            wvl = wv_p.tile([128, NKT, 512], BF, tag="wvl")
            nc.sync.dma_start(
                wvh[:], wvh_d[:, nh * 512:(nh + 1) * 512].rearrange(
                    "(t p) m -> p t m", p=128))
            nc.sync.dma_start(
                wvl[:], wvl_d[:, nh * 512:(nh + 1) * 512].rearrange(
                    "(t p) m -> p t m", p=128))
            return wvh, wvl

        # first strip's weights before the bulk hidden loads: PE starts sooner
        wstrip = load_w_strip(0)

        # resident transposed hidden halves: [128, NKT, S] per (b, hi/lo)
        hiT = []
        loT = []
        for b in range(BPC):
            th = hid_p.tile([128, NKT, S], BF, tag=f"hiT{b}")
            tl = hid_p.tile([128, NKT, S], BF, tag=f"loT{b}")
            nc.sync.dma_start(th[:], hiT_d[b].rearrange("(t p) n -> p t n", p=128))
            nc.sync.dma_start(tl[:], loT_d[b].rearrange("(t p) n -> p t n", p=128))
            hiT.append(th)
            loT.append(tl)

        wv_cur = [load_wv(0)]

        if mask_nz:
            msk = []
            for b in range(BPC):
                mt = msk_p.tile([128, S], F32, tag=f"msk{b}")
                nc.sync.dma_start(mt[:], msk_d[b])
                msk.append(mt)

        tq = [[None] * NSTRIP for _ in range(BPC)]
        tk = [[None] * NSTRIP for _ in range(BPC)]

        def proj_psum(ps, wh, wl, b):
            """accumulate x @ W for strip: 3-term bf16 split, W stationary."""
            for kt in range(NKT):
                st = kt == 0
                nc.tensor.matmul(ps[:], wh[:, kt, :], hiT[b][:, kt, :],
                                 start=st, stop=False)
                nc.tensor.matmul(ps[:], wh[:, kt, :], loT[b][:, kt, :],
                                 start=False, stop=False)
                nc.tensor.matmul(ps[:], wl[:, kt, :], hiT[b][:, kt, :],
                                 start=False, stop=(kt == NKT - 1))

        def ternarize(ps, lo_thr, hi_thr, out_bf):
            """out = Sign(ps + lo_thr) + Sign(ps + hi_thr)  in {-2,0,2} bf16."""
            a = ab_p.tile([128, S], BF, tag="a")
            bb = ab_p.tile([128, S], BF, tag="b")
            nc.scalar.activation(a[:], ps[:], AF.Sign, bias=lo_thr)
            nc.scalar.activation(bb[:], ps[:], AF.Sign, bias=hi_thr)
            nc.vector.tensor_add(out_bf[:], a[:], bb[:])

        # ---- V-phase work generator: one group = one (nh, b, tok-tile) ----
        def emit_v_group(g):
            nh, b, mt = g // 8, (g // 4) % 2, g % 4
            if g == 6:
                wv_cur.append(load_wv(1))      # prefetch second half of Wv
            wvh, wvl = wv_cur[-1] if g >= 8 else wv_cur[0]
            psv = ps_v.tile([128, 512], F32, tag="psv")
            sl = slice(mt * 128, (mt + 1) * 128)
            for kt in range(NKT):
                nc.tensor.matmul(psv[:], hiT[b][:, kt, sl], wvh[:, kt, :],
                                 start=(kt == 0), stop=False)
                nc.tensor.matmul(psv[:], hiT[b][:, kt, sl], wvl[:, kt, :],
                                 start=False, stop=False)
                last = (kt == NKT - 1) and not bias_nz
                nc.tensor.matmul(psv[:], loT[b][:, kt, sl], wvh[:, kt, :],
                                 start=False, stop=last)
            if bias_nz:
                nc.tensor.matmul(psv[:], ones_t[:],
                                 bvh_t[:, nh * 512:(nh + 1) * 512],
                                 start=False, stop=False)
                nc.tensor.matmul(psv[:], ones_t[:],
                                 bvl_t[:, nh * 512:(nh + 1) * 512],
                                 start=False, stop=True)
            tv = tv_p.tile([128, 512], BF, tag="tv")
            ternarize(psv, c_m[:], c_p[:], tv)
            vst = vq_p.tile([128, 512], F32, tag="vq")
            nc.vector.tensor_scalar(vst[:], tv[:], 1.25, None,
                                    mybir.AluOpType.mult)
            dview = vq_d[b, nh * 8:(nh + 1) * 8,
                         mt * 128:(mt + 1) * 128, :].rearrange("h p d -> p h d")
            nc.sync.dma_start(dview, vst[:].rearrange("p (h d) -> p h d", h=8))

        # ---------- Phase A: q/k projections + quantize, per strip ----------
        for s in range(NSTRIP):
            wqh, wql, wkh, wkl, thr_t = wstrip
            if s + 1 < NSTRIP:
                wstrip = load_w_strip(s + 1)   # prefetch next strip's weights

            for b in range(BPC):
                psq = ps_proj.tile([128, S], F32, tag="pp")
                proj_psum(psq, wqh, wql, b)
                tqt = tern_p.tile([128, S], BF, tag=f"tq{b}_{s}")
                ternarize(psq, thr_t[:, 0:1], thr_t[:, 1:2], tqt)
                tq[b][s] = tqt

                psk = ps_proj.tile([128, S], F32, tag="pp")
                proj_psum(psk, wkh, wkl, b)
                tkt = tern_p.tile([128, S], BF, tag=f"tk{b}_{s}")
                ternarize(psk, thr_t[:, 2:3], thr_t[:, 3:4], tkt)
                tk[b][s] = tkt

            # ------- scores + softmax for the 2 heads of this strip -------
            for b in range(BPC):
                for hh in range(2):
                    h = 2 * s + hh
                    off = hh * D
                    sc_t = sc_p.tile([128, NQT, S], F32, tag="sc")
                    p_t = p_p.tile([128, NQT, S], F32, tag="p")
                    ssum = sum_p.tile([128, NQT], F32, tag="ssum")
                    rinv = sum_p.tile([128, NQT], F32, tag="rinv")
                    for m in range(NQT):
                        pss = ps_sc.tile([128, S], F32, tag="pss")
                        nc.tensor.matmul(
                            pss[:],
                            tq[b][s][off:off + D, m * 128:(m + 1) * 128],
                            tk[b][s][off:off + D, :],
                            start=True, stop=True)
                        if mask_nz:
                            # scores = SCALE*psum + mask ; exp from SBUF scores
                            nc.vector.tensor_scalar(
                                sc_t[:, m, :], pss[:], SCALE, None,
                                mybir.AluOpType.mult)
                            nc.vector.tensor_add(sc_t[:, m, :], sc_t[:, m, :],
                                                 msk[b][:])
                            nc.scalar.activation(
                                p_t[:, m, :], sc_t[:, m, :], AF.Exp,
                                accum_out=ssum[:, m:m + 1])
                        else:
                            nc.vector.tensor_scalar(
                                sc_t[:, m, :], pss[:], SCALE, None,
                                mybir.AluOpType.mult)
                            nc.scalar.activation(
                                p_t[:, m, :], pss[:], AF.Exp, scale=SCALE,
                                accum_out=ssum[:, m:m + 1])
                    nc.vector.reciprocal(rinv[:], ssum[:])
                    for m in range(NQT):
                        nc.vector.tensor_scalar(
                            p_t[:, m, :], p_t[:, m, :], rinv[:, m:m + 1], None,
                            mybir.AluOpType.mult)
                    dview = sc_d[b, h].rearrange("(m p) k -> p m k", p=128)
                    nc.sync.dma_start(dview, sc_t[:])
                    nc.sync.dma_start(
                        pr_d[b, h].rearrange("(m p) k -> p m k", p=128), p_t[:])

            # interleave two V-phase groups per strip to fill PE gaps
            emit_v_group(2 * s)
            emit_v_group(2 * s + 1)

    nc.compile()
    return nc


def _get_nc(bias_nz: bool, mask_nz: bool):
    key = (bias_nz, mask_nz)
    if key not in _CACHE:
        _CACHE[key] = _build(bias_nz, mask_nz)
    return _CACHE[key]


def _split_hi_lo(x32):
    hi = x32.astype(BF16)
    lo = (x32 - hi.astype(np.float32)).astype(BF16)
    return hi, lo


def _prep(hidden_states, attention_mask, Wq, bq, Wk, bk, Wv, bv):
    hs = np.asarray(hidden_states, np.float32)
    msk = np.asarray(attention_mask, np.float32)
    Wq = np.asarray(Wq, np.float32)
    Wk = np.asarray(Wk, np.float32)
    Wv = np.asarray(Wv, np.float32)
    bq = np.asarray(bq, np.float32)
    bk = np.asarray(bk, np.float32)
    bv = np.asarray(bv, np.float32)

    bias_nz = bool(np.any(bq) or np.any(bk) or np.any(bv))
    mask_nz = bool(np.any(msk))

    # transposed hidden, hi/lo split: [B, H, S]
    hsT = np.ascontiguousarray(hs.transpose(0, 2, 1))
    hiT, loT = _split_hi_lo(hsT)
    wqh, wql = _split_hi_lo(Wq)
    wkh, wkl = _split_hi_lo(Wk)
    wvh, wvl = _split_hi_lo(Wv)
    thr = np.stack([bq - 1.25, bq + 1.25, bk - 1.25, bk + 1.25],
                   axis=1).astype(np.float32)

    shared = {
        "wqh": np.ascontiguousarray(wqh), "wql": np.ascontiguousarray(wql),
        "wkh": np.ascontiguousarray(wkh), "wkl": np.ascontiguousarray(wkl),
        "wvh": np.ascontiguousarray(wvh), "wvl": np.ascontiguousarray(wvl),
        "thr": thr,
    }
    if bias_nz:
        bvh, bvl = _split_hi_lo(bv)
        shared["bvh"] = np.ascontiguousarray(bvh[None, :])
        shared["bvl"] = np.ascontiguousarray(bvl[None, :])

    in_maps = []
    for c in range(N_CORES):
        bsl = slice(c * BPC, (c + 1) * BPC)
        m = dict(shared)
        m["hiT"] = np.ascontiguousarray(hiT[bsl])
        m["loT"] = np.ascontiguousarray(loT[bsl])
        if mask_nz:
            mm = msk[bsl, 0, 0, :]                      # [BPC, S]
            m["msk"] = np.ascontiguousarray(
                np.broadcast_to(mm[:, None, :], (BPC, 128, S)).astype(np.float32))
        in_maps.append(m)
    return in_maps, bias_nz, mask_nz


def _run(hidden_states, attention_mask, Wq, bq, Wk, bk, Wv, bv, trace=False):
    in_maps, bias_nz, mask_nz = _prep(hidden_states, attention_mask,
                                      Wq, bq, Wk, bk, Wv, bv)
    nc = _get_nc(bias_nz, mask_nz)
    res = bass_utils.run_bass_kernel_spmd(
        nc, in_maps, core_ids=list(range(N_CORES)), trace=trace)

    scores = np.concatenate([res.results[c]["scores"] for c in range(N_CORES)], 0)
    probs = np.concatenate([res.results[c]["probs"] for c in range(N_CORES)], 0)
    vq = np.concatenate([res.results[c]["vq"] for c in range(N_CORES)], 0)

    context_layer = np.zeros((B, S, H), np.float32)
    context_layer_ = np.zeros((B, NH, S, D), np.float32)
    out = (context_layer, scores, probs, context_layer_, vq)
    return (out, res) if trace else out


def kernel(hidden_states, attention_mask, Wq, bq, Wk, bk, Wv, bv):
    return _run(hidden_states, attention_mask, Wq, bq, Wk, bk, Wv, bv)
